# revision 52
# baseline (speedup 1.0000x reference)
"""Multi-head attention (B=2, S=4096, D=768, H=12) on 8 Trainium2 cores.

Sharding: core c -> batch b = c // 4, head-triple g = c % 4 (heads 3g..3g+2).
Each core computes its QKV projections (columns of W for its heads) and
flash-style attention for its 3 heads, fully on-chip; no cross-core comms.
Host-side prep per core: slice batch/head-group, cast x/W to bf16 (the device
kernel computes in bf16 with f32 accumulation either way; casting host-side
also halves the transfer volume).

Per-core device kernel:
  - x^T tiles [128(d), 6(dchunk), 1024(s)] via xbar DMA-transpose straight
    from the bf16 DRAM inputs, quarter by quarter; projections chase each
    quarter so ScalarE attention work starts ~tens of us in.
  - projections on PE produce qT/kT [64, 3, 4096] and v_ext
    [128, 32, 3, 65] (col 64 = ones so the PV matmul accumulates the softmax
    denominator as output row 64). The attention mask enters as a per-k scale
    em = exp(-1e4*(1-mask)) folded into v_ext (exact: softmax with additive
    -1e4 adder == scaling exp(s) by em, including the denominator).
  - attention per (head, 512-wide q chunk): 32 k-chunks in groups of 3;
    QK^T -> PSUM, exp on ScalarE (scale=1/8) -> bf16 probs in SBUF,
    PV accumulate -> PSUM [65, 512]; then PE-transpose to natural layout and
    DVE normalize by the reciprocal of the denominator column (row 64).
"""

import sys

if "/opt/trn_rl_repo" not in sys.path:
    sys.path.insert(0, "/opt/trn_rl_repo")

from contextlib import ExitStack

import ml_dtypes
import numpy as np

import concourse.bass as bass
import concourse.tile as tile
from concourse import bacc, mybir
from concourse.bass_utils import run_bass_kernel_spmd
from concourse.masks import make_identity

F32 = mybir.dt.float32
# fp16 instead of bf16: all on-chip value ranges here are tiny (|x|<6,
# |W|<0.12, probs<8), so fp16's 10 mantissa bits cut quantization error ~4x
# at identical PE throughput (1 cycle/row) and xbar 2-byte transpose support
BF16 = mybir.dt.float16
AF = mybir.ActivationFunctionType
ALU = mybir.AluOpType
BF16_NP = np.float16

B, S, D, H, DK = 2, 4096, 768, 12, 64
N_CORES = 8
HPG = 3            # heads per core
GD = HPG * DK      # 192 output columns per core
SQ = 512           # q-chunk width
NSQ = S // SQ      # 8
KCW = 128          # k-chunk width
NKC = S // KCW     # 32
GRP = 3            # k-chunks per exp group (3 PSUM banks, double buffered)
NDC = D // 128     # 6 contraction chunks
QTR = S // 4       # transpose/projection pipeline granularity
SQQ = NSQ // 4     # q chunks per quarter
SCQ = NKC // 4     # s chunks per quarter


def _emit(ctx: ExitStack, tc: tile.TileContext, io: dict):
    nc = tc.nc

    const = ctx.enter_context(tc.tile_pool(name="const", bufs=1))
    xt_pool = ctx.enter_context(tc.tile_pool(name="xt", bufs=5))
    proj = ctx.enter_context(tc.tile_pool(name="proj", bufs=1))
    scores_pool = ctx.enter_context(tc.tile_pool(name="scores", bufs=2, space="PSUM"))
    aux_psum = ctx.enter_context(tc.tile_pool(name="auxp", bufs=2, space="PSUM"))
    probs_pool = ctx.enter_context(tc.tile_pool(name="probs", bufs=5))
    outt_pool = ctx.enter_context(tc.tile_pool(name="outt", bufs=2))
    small = ctx.enter_context(tc.tile_pool(name="small", bufs=2))
    oslab_pool = ctx.enter_context(tc.tile_pool(name="oslab", bufs=3))

    # ---- constants / small inputs (consolidated to limit 4KB slot padding) ----
    # mask -> per-k scale em = exp(-1e4 * (1 - mask)), [128, 32] (p, kchunk).
    # Emitted FIRST so the ACT exp-table load lands at the head of the queues.
    mask_em = const.tile([128, 65], F32, name="mask_em")
    mask_t = mask_em[:, 0:32]
    em_sb = mask_em[:, 32:64]
    neg1e4 = mask_em[:, 64:65]
    nc.gpsimd.memset(neg1e4, -10000.0)
    nc.scalar.dma_start(mask_t, io["mask_pk"][:])
    nc.scalar.activation(em_sb, mask_t, AF.Exp, scale=10000.0, bias=neg1e4)

    # weights loaded contiguously (q | k | v along free dim)
    w_all = const.tile([128, NDC, 3 * GD], BF16, name="w_all")
    for i, nm in ((1, "wk"), (0, "wq"), (2, "wv")):
        nc.scalar.dma_start(
            w_all[:, :, i * GD : (i + 1) * GD],
            io[nm].rearrange("(dc p) n -> p dc n", p=128),
        )
    wv_sb = w_all[:, :, 2 * GD : 3 * GD]

    # q/k weights with each head's 64 columns duplicated (projection then
    # replicates qT/kT on both partition halves at no extra PE cost)
    w_dup = const.tile([128, NDC, 2, HPG, 128], BF16, name="w_dup")
    for i in (1, 0):
        for h in range(HPG):
            for rep in range(2):
                nc.vector.tensor_copy(
                    w_dup[:, :, i, h, rep * DK : (rep + 1) * DK],
                    w_all[:, :, i * GD + h * DK : i * GD + (h + 1) * DK],
                )

    bqbk = const.tile([128, 2 * HPG], F32, name="bqbk")
    nc.scalar.dma_start(bqbk[:], io["bqbk_pk"][:])

    bfpack = const.tile([1, 320], BF16, name="bfpack")
    nc.gpsimd.memset(bfpack[:, 0:128], 1.0)
    nc.scalar.dma_start(bfpack[:, 128 : 128 + GD], io["bv_r"][:])
    ones_row = bfpack[:, 0:128]
    bv_sb = bfpack[:, 128 : 128 + GD]

    ident = const.tile([128, 128], F32, name="ident")
    make_identity(nc, ident[:])

    # ---- persistent projection outputs (qT/kT replicated on both halves) ----
    qT = proj.tile([128, HPG, S], BF16, name="qT")
    kT = proj.tile([128, HPG, S], BF16, name="kT")
    vE = proj.tile([128, NKC, HPG, DK + 1], BF16, name="vE")
    nc.gpsimd.memset(vE[:], 1.0)  # ones col 64; data cols overwritten below

    # ---- per-quarter: transpose + project ----
    def load_xt_quarter(nm, qq):
        # host supplies x d-chunk-major [6*4096, 128] so each xbar transpose
        # reads a fully contiguous [1024, 128] block
        xt = xt_pool.tile([128, NDC, QTR], BF16, tag="xt", name=f"xt_{nm}_{qq}")
        for dc in range(NDC):
            base = dc * S + qq * QTR
            nc.sync.dma_start(
                out=xt[:, dc, :], in_=io[nm][base : base + QTR, :],
                transpose=True,
            )
        return xt

    def proj_qk(xt, qq, wi, bias, dst):
        for h in range(HPG):
            for sqq in range(SQQ):
                sq = qq * SQQ + sqq
                ps = aux_psum.tile([128, SQ], F32, tag="aux", name=f"ps_{qq}_{h}_{sqq}")
                for dc in range(NDC):
                    nc.tensor.matmul(
                        ps[:],
                        lhsT=w_dup[:, dc, wi, h, :],
                        rhs=xt[:, dc, sqq * SQ : (sqq + 1) * SQ],
                        start=(dc == 0),
                        stop=(dc == NDC - 1),
                    )
                nc.vector.tensor_scalar(
                    dst[:, h, sq * SQ : (sq + 1) * SQ], ps[:],
                    bias[:, h : h + 1], None, ALU.add,
                )

    def proj_v(xt, qq):
        for scq in range(SCQ):
            sc = qq * SCQ + scq
            ps = aux_psum.tile([128, GD], F32, tag="aux", name=f"psv_{qq}_{scq}")
            for dc in range(NDC):
                nc.tensor.matmul(
                    ps[:],
                    lhsT=xt[:, dc, scq * 128 : (scq + 1) * 128],
                    rhs=wv_sb[:, dc, :],
                    start=(dc == 0),
                    stop=False,
                )
            nc.tensor.matmul(
                ps[:], lhsT=ones_row[:, 0:128], rhs=bv_sb[:], start=False, stop=True
            )
            for h in range(HPG):
                nc.vector.tensor_copy(
                    vE[:, sc, h, 0:DK], ps[:, h * DK : (h + 1) * DK]
                )
            # fold mask scale into v and the denominator ones column
            nc.vector.tensor_scalar(
                vE[:, sc, :, :], vE[:, sc, :, :], em_sb[:, sc : sc + 1], None,
                ALU.mult,
            )

    def proj_kv_quarter(qq):
        xt_k = load_xt_quarter("xk", qq)
        proj_qk(xt_k, qq, 1, bqbk[:, HPG : 2 * HPG], kT)
        xt_v = load_xt_quarter("xv", qq)
        proj_v(xt_v, qq)

    def proj_q_group(xt, qq, h, sqq):
        # borrows a scores-pool slot: the aux pool's two slots hold live pv
        # accumulators / tr tiles during attention (a third tenant would
        # deadlock the in-order PE queue on slot reuse)
        sq = qq * SQQ + sqq
        ps = scores_pool.tile([128, SQ], F32, tag="scores", name=f"psq_{qq}_{h}_{sqq}")
        for dc in range(NDC):
            nc.tensor.matmul(
                ps[:],
                lhsT=w_dup[:, dc, 0, h, :],
                rhs=xt[:, dc, sqq * SQ : (sqq + 1) * SQ],
                start=(dc == 0),
                stop=(dc == NDC - 1),
            )
        nc.vector.tensor_scalar(
            qT[:, h, sq * SQ : (sq + 1) * SQ], ps[:],
            bqbk[:, h : h + 1], None, ALU.add,
        )

    # ---- attention ----
    groups = []
    g0 = 0
    while g0 < NKC:
        groups.append((g0, min(GRP, NKC - g0)))
        g0 += GRP

    pending = None  # finalize closure for the previous (h, sq)

    def make_finalize(pv, h, sq):
        def fin():
            ot = outt_pool.tile([DK + 1, SQ], F32, tag="outt", name=f"ot_{h}_{sq}")
            nc.vector.tensor_copy(ot[:], pv[:])
            tr = aux_psum.tile([128, 4 * (DK + 1)], F32, tag="aux", name=f"tr_{h}_{sq}")
            for t in range(4):
                nc.tensor.transpose(
                    tr[:, t * (DK + 1) : (t + 1) * (DK + 1)],
                    ot[:, t * 128 : (t + 1) * 128],
                    ident[0 : DK + 1, 0 : DK + 1],
                )
            rc = small.tile([128, 4], F32, tag="recip", name=f"rc_{h}_{sq}")
            osl = oslab_pool.tile([128, 4, DK], F32, tag="oslab", name=f"os_{h}_{sq}")
            for t in range(4):
                nc.vector.reciprocal(
                    rc[:, t : t + 1], tr[:, t * (DK + 1) + DK : t * (DK + 1) + DK + 1]
                )
                nc.vector.tensor_scalar(
                    osl[:, t, :],
                    tr[:, t * (DK + 1) : t * (DK + 1) + DK],
                    rc[:, t : t + 1],
                    None,
                    ALU.mult,
                )
            nc.gpsimd.dma_start(
                out=io["out"].rearrange(
                    "(sq t p) n -> sq p t n", sq=NSQ, t=4, p=128
                )[sq, :, :, h * DK : (h + 1) * DK],
                in_=osl[:],
            )
        return fin

    # Boundary tasks: q-quarter transposes + projection groups for sq>=2 are
    # deferred into the attention phase (PE has per-group slack there), keyed
    # by the flat iteration index after which they are emitted.
    boundary_tasks = {}
    xt_q_tiles = {}

    def sched(it, fn):
        boundary_tasks.setdefault(it, []).append(fn)

    def tr_q(qq):
        def fn():
            xt_q_tiles[qq] = load_xt_quarter("xq", qq)
        return fn

    def pg(qq, h, sqq):
        def fn():
            proj_q_group(xt_q_tiles[qq], qq, h, sqq)
        return fn

    # task scheduled at boundary b fires during iteration b+1 (after its first
    # exp group), so pg for (h, sq) must sit at boundary <= idx(h, sq) - 2
    DEFER_PG = __import__("os").environ.get("BASS_DEFER_PG", "0") == "1"
    if DEFER_PG:
        sched(0, pg(1, 0, 0))
        sched(1, pg(1, 0, 1))
        sched(2, pg(2, 0, 0))
        sched(3, pg(2, 0, 1))
        sched(4, pg(3, 0, 0))
        sched(5, pg(3, 0, 1))
        nb = 6
        for h in (1, 2):
            for qq in (1, 2, 3):
                for sqq in range(SQQ):
                    sched(nb, pg(qq, h, sqq))
                    nb += 1

    def attention_gen():
        nonlocal_pending = [None]

        def emit_pv(pv, h, grp):
            p0, plen, ppr = grp
            for j in range(plen):
                kc = p0 + j
                nc.tensor.matmul(
                    pv[:],
                    lhsT=vE[:, kc, h, :],
                    rhs=ppr[:, j * SQ : (j + 1) * SQ],
                    start=(kc == 0),
                    stop=(kc == NKC - 1),
                )

        carry = None  # (pv, h, [groups]) tail-PV work carried across iterations
        it = 0
        for h in range(HPG):
            for sq in range(NSQ):
                pv = aux_psum.tile([DK + 1, SQ], F32, tag="aux", name=f"pv_{h}_{sq}")
                sc_t = [None] * len(groups)
                filled = [0] * len(groups)
                ready = []  # (kc0, glen, probs) groups awaiting PV emission
                for kc in range(NKC):
                    gi, j = kc // GRP, kc % GRP
                    kc0, glen = groups[gi]
                    if sc_t[gi] is None:
                        sc_t[gi] = scores_pool.tile(
                            [128, glen * SQ], F32, tag="scores",
                            name=f"sc_{h}_{sq}_{gi}",
                        )
                    nc.tensor.matmul(
                        sc_t[gi][:, j * SQ : (j + 1) * SQ],
                        lhsT=kT[0:DK, h, kc * KCW : (kc + 1) * KCW],
                        rhs=qT[0:DK, h, sq * SQ : (sq + 1) * SQ],
                        start=True,
                        stop=True,
                    )
                    filled[gi] += 1
                    if filled[gi] == glen:
                        pr = probs_pool.tile(
                            [128, glen * SQ], BF16, tag="probs",
                            name=f"pr_{h}_{sq}_{gi}",
                        )
                        nc.scalar.activation(pr[:], sc_t[gi][:], AF.Exp, scale=0.125)
                        ready.append((kc0, glen, pr))
                        if gi == 0 and carry is not None:
                            cpv, ch, cgrps = carry
                            for grp in cgrps:
                                emit_pv(cpv, ch, grp)
                            carry = None
                            for fn in boundary_tasks.get(it - 1, ()):
                                fn()
                        if gi == 1 and nonlocal_pending[0] is not None:
                            nonlocal_pending[0]()
                            nonlocal_pending[0] = None
                        if len(ready) >= 2:
                            emit_pv(pv, h, ready.pop(0))
                        yield (h, sq, gi)
                carry = (pv, h, list(ready))
                nonlocal_pending[0] = make_finalize(pv, h, sq)
                it += 1

        cpv, ch, cgrps = carry
        for grp in cgrps:
            emit_pv(cpv, ch, grp)
        nonlocal_pending[0]()

    # Interleave k/v projection quarters with the first attention iteration's
    # k-chunk groups so ScalarE saturates early: group gi covers kc
    # 3*gi..3*gi+2, requiring k/v quarters up to (3*gi+2)//8; the first
    # iteration uses q chunk sq=0 (quarter 0).
    gen = attention_gen()

    def advance(n):
        for _ in range(n):
            if next(gen, None) is None:
                break

    # quarter 0 in k, q, v order: the first QK group needs kT+qT only (the
    # first PV trails by two exp groups, so v can land a little later)
    xt_k0 = load_xt_quarter("xk", 0)
    proj_qk(xt_k0, 0, 1, bqbk[:, HPG : 2 * HPG], kT)
    xt_q_tiles[0] = load_xt_quarter("xq", 0)
    for h in range(HPG):
        for sqq in range(SQQ):
            proj_q_group(xt_q_tiles[0], 0, h, sqq)
    advance(1)       # g0: kc 0..2 (needs only kT+qT of quarter 0)
    xt_v0 = load_xt_quarter("xv", 0)
    proj_v(xt_v0, 0)
    advance(1)       # g1: kc 3..5 (first PV fires after this exp)
    proj_kv_quarter(1)
    xt_q_tiles[1] = load_xt_quarter("xq", 1)
    if not DEFER_PG:
        for h in range(HPG):
            for sqq in range(SQQ):
                proj_q_group(xt_q_tiles[1], 1, h, sqq)
    advance(3)       # g2..g4: kc 6..14 (quarters 0-1)
    proj_kv_quarter(2)
    xt_q_tiles[2] = load_xt_quarter("xq", 2)
    if not DEFER_PG:
        for h in range(HPG):
            for sqq in range(SQQ):
                proj_q_group(xt_q_tiles[2], 2, h, sqq)
    advance(3)       # g5..g7: kc 15..23 (quarter 2)
    proj_kv_quarter(3)
    xt_q_tiles[3] = load_xt_quarter("xq", 3)
    if not DEFER_PG:
        for h in range(HPG):
            for sqq in range(SQQ):
                proj_q_group(xt_q_tiles[3], 3, h, sqq)
    for _ in gen:
        pass


def _build():
    nc = bacc.Bacc("TRN2", target_bir_lowering=False, debug=False)
    io = {}
    for nm, shape, dt in (
        ("xq", [NDC * S, 128], BF16), ("xk", [NDC * S, 128], BF16),
        ("xv", [NDC * S, 128], BF16),
        ("wq", [D, GD], BF16), ("wk", [D, GD], BF16), ("wv", [D, GD], BF16),
        ("bqbk_pk", [128, 2 * HPG], F32),
        ("bv_r", [1, GD], BF16), ("mask_pk", [128, NKC], F32),
    ):
        io[nm] = nc.dram_tensor(nm, shape, dt, kind="ExternalInput").ap()
    io["out"] = nc.dram_tensor("out", [S, GD], F32, kind="ExternalOutput").ap()

    import os

    dup = int(os.environ.get("BASS_DUP", "1"))
    with tile.TileContext(nc) as tc:
        for _ in range(dup):
            with ExitStack() as ctx:
                _emit(ctx, tc, io)
    nc.compile()
    return nc


_NC = None


def _get_nc():
    global _NC
    if _NC is None:
        _NC = _build()
    return _NC


def make_in_maps(query, key, value, mask, Wq, bq, Wk, bk, Wv, bv):
    bf = lambda a: np.ascontiguousarray(a).astype(BF16_NP)
    bf3 = lambda a: np.ascontiguousarray(
        np.asarray(a).reshape(S, NDC, 128).transpose(1, 0, 2).reshape(NDC * S, 128)
    ).astype(BF16_NP)
    f32 = lambda a: np.ascontiguousarray(np.asarray(a, np.float32))
    in_maps = []
    for c in range(N_CORES):
        b, g = divmod(c, 4)
        cols = slice(g * GD, (g + 1) * GD)
        in_maps.append({
            "xq": bf3(query[b]),
            "xk": bf3(key[b]),
            "xv": bf3(value[b]),
            "wq": bf(Wq[:, cols]),
            "wk": bf(Wk[:, cols]),
            "wv": bf(Wv[:, cols]),
            "bqbk_pk": f32(np.tile(np.concatenate(
                [np.asarray(bq)[cols].reshape(HPG, DK).T,
                 np.asarray(bk)[cols].reshape(HPG, DK).T], axis=1), (2, 1))),
            "bv_r": bf(np.asarray(bv)[cols].reshape(1, GD)),
            "mask_pk": f32(np.asarray(mask)[b].reshape(NKC, 128).T),
        })
    return in_maps


def kernel(query, key, value, mask, Wq, bq, Wk, bk, Wv, bv):
    query = np.asarray(query, np.float32)
    key = np.asarray(key, np.float32)
    value = np.asarray(value, np.float32)
    nc = _get_nc()
    in_maps = make_in_maps(query, key, value, mask, Wq, bq, Wk, bk, Wv, bv)
    res = run_bass_kernel_spmd(nc, in_maps, core_ids=list(range(N_CORES)))
    out = np.empty((B, S, D), np.float32)
    for c in range(N_CORES):
        b, g = divmod(c, 4)
        out[b, :, g * GD : (g + 1) * GD] = res.results[c]["out"]
    return out
